# revision 20
# baseline (speedup 1.0000x reference)
"""Trainium2 Bass kernel for nn_ActorNetwork (topk_masking).

Pure data-parallel over 8 NeuronCores: each core processes 8192 rows of the
65536-row batch (512-row tiles, 4x128-row subtiles).

Math per row b, customer j (C=512):
  h1 = relu(h @ W1 + b1), h2 = relu(h1 @ W2 + b2)      h = [x|y|d|uav]
  logits = h2 @ Wr + br; maintenance = sigmoid(h2 @ Wm + bm)
  mask: col0 = dest != 0;  col 1+j = seq_j & (load+dem_j <= ML) & (e_wh > 0)
    e_wh = energy - cost*s,  cost = ((ep*x^1.5)*0.1)/ME,
    x = (100+load)+dem_j,  s = dist[dest,1+j]/fv + dist[1+j,0]/fv
  probs = softmax(where(mask, logits, -1e9))

Device strategy:
  * MLP in transposed layout (bf16 matmuls, fp32 psum), heads in natural
    layout; b1/b2 via per-partition ACT bias, br/bm folded into the penalty
    plane / epilogue constants.
  * masking via additive -1e9 penalties folded into logits before exp.
  * feasibility: rows with energy > sup(cost*s) (fp64 bound) are feasible at
    every j -- the host groups the few "hard" rows into leading tiles; only
    those run the exact fp32 squared comparison
        s^2 * x^3 < energy^2 / (ep*0.1/ME)^2
    (multiplies only -- bit-faithful to ~3ulp of the reference boundary)
    with s gathered from a host-precomputed table via dma_gather.
  * softmax without max-subtraction (logits are O(0.1) by construction);
    exp and row-sum fused in one ACT op (accum_out); 1/sum scale split
    between ACT Copy and DVE. Only the exp_and_others ACT table set is used
    (sigmoid computed as 1/(1+exp(-v))).
  * software-pipelined: tile t's trunk (h1/h2 matmuls) is issued interleaved
    with tile t-1's heads/mask/softmax so every engine FIFO always has ready
    work; loads issue on Sync HWDGE, stores on GpSimd SWDGE.

Host does only relabeling (transpose/cast/permute/concat) plus O(C^2)
constant tables and O(B) row classification.
"""

import os
import sys

for _p in ("/opt/trn_rl_repo", "/root/.axon_site/_ro/trn_rl_repo", "/root/.axon_site"):
    if os.path.isdir(_p) and _p not in sys.path:
        sys.path.append(_p)

os.environ.setdefault("JAX_PLATFORMS", "axon")

import numpy as np
import ml_dtypes

import concourse.bass as bass
import concourse.mybir as mybir
import concourse.tile as tile
from concourse import bacc
from concourse.bass import ts
from concourse.bass_utils import run_bass_kernel_spmd

F32 = mybir.dt.float32
BF16 = mybir.dt.bfloat16
I16 = mybir.dt.int16
AF = mybir.ActivationFunctionType
OP = mybir.AluOpType

N_CORES = 8
B = 65536
C = 512
OBS = 519
DIN = OBS + 3 * C            # 2055
BSH = B // N_CORES           # 8192
NT = BSH // 512              # 16 tiles of 512 rows
KFULL = DIN // 128           # 16 full K-chunks
KREM = DIN - KFULL * 128     # 7
NSUB = 4                     # 4 subtiles of 128 rows per tile
NBIG = KFULL + NSUB          # 20: ht chunks + seqpen planes in one buffer

_PROG_CACHE = {}
LAST_RESULT = None


def _build_program(inv_c1sq, n_hard, eb0, embm):
    nc = bacc.Bacc("TRN2", target_bir_lowering=False, debug=False)

    big_d = nc.dram_tensor("big", [128, NT, NBIG, 512], BF16, kind="ExternalInput")
    ht7_d = nc.dram_tensor("ht7", [KREM, NT, 512], BF16, kind="ExternalInput")
    msc_d = nc.dram_tensor("msc", [128, NT, NSUB, 4], F32, kind="ExternalInput")
    didx_d = nc.dram_tensor("didx", [128, max(n_hard, 1) * 32], I16,
                            kind="ExternalInput")
    tab_d = nc.dram_tensor("tab", [C + 1, C], F32, kind="ExternalInput")
    w1_d = nc.dram_tensor("w1", [DIN, 128], BF16, kind="ExternalInput")
    w2_d = nc.dram_tensor("w2", [128, 128], BF16, kind="ExternalInput")
    wrm_d = nc.dram_tensor("wrm", [128, C + 2], BF16, kind="ExternalInput")
    dem_d = nc.dram_tensor("dem", [1, C], F32, kind="ExternalInput")
    b1_d = nc.dram_tensor("b1v", [128, 1], F32, kind="ExternalInput")
    b2_d = nc.dram_tensor("b2v", [128, 1], F32, kind="ExternalInput")

    probs_d = nc.dram_tensor("probs", [128, NT, NSUB, C + 1], F32,
                             kind="ExternalOutput")
    maint_d = nc.dram_tensor("maint", [128, NT, NSUB], F32, kind="ExternalOutput")

    with tile.TileContext(nc) as tc:
        with tc.tile_pool(name="const", bufs=1) as cpool:
            w1sb = cpool.tile([128, KFULL + 1, 128], BF16)
            nc.sync.dma_start(
                w1sb[:, 0:KFULL, :],
                w1_d.ap()[0 : KFULL * 128, :].rearrange("(k p) m -> p k m", p=128),
            )
            nc.sync.dma_start(w1sb[0:KREM, KFULL, :], w1_d.ap()[KFULL * 128 :, :])
            w2sb = cpool.tile([128, 128], BF16)
            nc.sync.dma_start(w2sb[:], w2_d.ap())
            wrmsb = cpool.tile([128, C + 2], BF16)
            nc.sync.dma_start(wrmsb[:], wrm_d.ap())
            b1sb = cpool.tile([128, 1], F32)
            nc.sync.dma_start(b1sb[:], b1_d.ap())
            b2sb = cpool.tile([128, 1], F32)
            nc.sync.dma_start(b2sb[:], b2_d.ap())
            dem1 = cpool.tile([1, C], F32)
            nc.sync.dma_start(dem1[:], dem_d.ap())
            demb = cpool.tile([128, C], F32)
            nc.gpsimd.partition_broadcast(demb[:], dem1[0:1, :])
            didxsb = cpool.tile([128, max(n_hard, 1) * 32], I16)
            nc.sync.dma_start(didxsb[:], didx_d.ap())

            with (
                tc.tile_pool(name="big", bufs=4) as p_big,
                tc.tile_pool(name="ht7", bufs=4) as p_ht7,
                tc.tile_pool(name="msc", bufs=4) as p_msc,
                tc.tile_pool(name="gat", bufs=1) as p_gat,
                tc.tile_pool(name="h12", bufs=4) as p_h12,
                tc.tile_pool(name="wrk", bufs=3) as p_wrk,
                tc.tile_pool(name="hwk", bufs=1) as p_hwk,
                tc.tile_pool(name="sml", bufs=3) as p_sml,
                tc.tile_pool(name="out", bufs=3) as p_out,
                tc.tile_pool(name="psA", bufs=2, space="PSUM") as psA,
                tc.tile_pool(name="psL", bufs=3, space="PSUM") as psL,
                tc.tile_pool(name="psS", bufs=2, space="PSUM") as psS,
            ):
                state = {}          # per-tile tiles carried trunk -> heads

                def emit_loads(t):
                    with tc.high_priority(offset=250):
                        bigsb = p_big.tile([128, NBIG, 512], BF16, tag="big")
                        nc.sync.dma_start(bigsb[:], big_d.ap()[:, t, :, :])
                        ht7sb = p_ht7.tile([128, 512], BF16, tag="ht7")
                        nc.sync.dma_start(ht7sb[0:KREM, :], ht7_d.ap()[:, t, :])
                        mscsb = p_msc.tile([128, NSUB, 4], F32, tag="msc")
                        nc.sync.dma_start(mscsb[:], msc_d.ap()[:, t, :, :])
                        ssb = None
                        if t < n_hard:
                            ssb = p_gat.tile([128, NSUB, C], F32, tag="gat")
                            nc.gpsimd.dma_gather(
                                out_ap=ssb[:], in_ap=tab_d.ap(),
                                idxs_ap=didxsb[:, ts(t, 32)],
                                num_idxs=512, num_idxs_reg=512, elem_size=C,
                            )
                    state[t] = dict(big=bigsb, ht7=ht7sb, msc=mscsb, s=ssb)

                def emit_h1(t):
                    st = state[t]
                    ph1 = psA.tile([128, 512], F32, tag="psA")
                    for k in range(KFULL):
                        nc.tensor.matmul(ph1[:], w1sb[:, k, :],
                                         st["big"][:, k, :],
                                         start=(k == 0), stop=False)
                    nc.tensor.matmul(ph1[:], w1sb[0:KREM, KFULL, :],
                                     st["ht7"][0:KREM, :], start=False, stop=True)
                    h1sb = p_h12.tile([128, 512], BF16, tag="h12")
                    nc.scalar.activation(h1sb[:], ph1[:], AF.Relu,
                                         bias=b1sb[:], scale=1.0)
                    st["h1"] = h1sb

                def emit_h2(t):
                    st = state[t]
                    ph2 = psA.tile([128, 512], F32, tag="psA")
                    nc.tensor.matmul(ph2[:], w2sb[:], st["h1"][:],
                                     start=True, stop=True)
                    h2sb = p_h12.tile([128, 512], BF16, tag="h12")
                    nc.scalar.activation(h2sb[:], ph2[:], AF.Relu,
                                         bias=b2sb[:], scale=1.0)
                    st["h2"] = h2sb

                def emit_heads(t):
                    st = state.pop(t)
                    hard = t < n_hard
                    bigsb, mscsb, h2sb, ssb = st["big"], st["msc"], st["h2"], st["s"]
                    outsb = p_out.tile([128, NSUB, C + 1], F32, tag="out")
                    rows4 = p_sml.tile([128, NSUB], F32, tag="rows4")
                    pl2x = psS.tile([128, 2 * NSUB], F32, tag="psS")

                    if hard:
                        l100 = p_sml.tile([128, NSUB], F32, tag="l100")
                        nc.vector.tensor_scalar(
                            out=l100[:], in0=mscsb[:, :, 0], scalar1=100.0,
                            scalar2=None, op0=OP.add)
                        e2t = p_sml.tile([128, NSUB], F32, tag="e2t")
                        nc.vector.tensor_tensor(
                            out=e2t[:], in0=mscsb[:, :, 1], in1=mscsb[:, :, 1],
                            op=OP.mult)
                        E2 = p_sml.tile([128, NSUB], F32, tag="E2")
                        nc.vector.tensor_scalar(
                            out=E2[:], in0=e2t[:], scalar1=float(inv_c1sq),
                            scalar2=None, op0=OP.mult)

                    for s in range(NSUB):
                        pl = psL.tile([128, 512], F32, tag="psL")
                        nc.tensor.matmul(pl[:], h2sb[:, ts(s, 128)],
                                         wrmsb[:, 0:512], start=True, stop=True)
                        nc.tensor.matmul(pl2x[:, 2 * s : 2 * s + 2],
                                         h2sb[:, ts(s, 128)],
                                         wrmsb[:, 512:514], start=True, stop=True)

                        # logits + seq penalty (-1e9 where seq==0, + br)
                        ml = p_wrk.tile([128, C], F32, tag="ml")
                        nc.vector.tensor_tensor(
                            out=ml[:], in0=pl[:], in1=bigsb[:, KFULL + s, :],
                            op=OP.add)

                        if hard:
                            xb = p_hwk.tile([128, C], F32, tag="xb")
                            nc.vector.tensor_scalar(
                                out=xb[:], in0=demb[:], scalar1=l100[:, s : s + 1],
                                scalar2=None, op0=OP.add)
                            sx = p_hwk.tile([128, C], F32, tag="sx")
                            nc.vector.tensor_tensor(
                                out=sx[:], in0=ssb[:, s, :], in1=xb[:], op=OP.mult)
                            sx2 = p_hwk.tile([128, C], F32, tag="sx2")
                            nc.scalar.activation(sx2[:], sx[:], AF.Square)
                            lhs = p_hwk.tile([128, C], F32, tag="lhs")
                            nc.gpsimd.tensor_tensor(
                                out=lhs[:], in0=sx2[:], in1=xb[:], op=OP.mult)
                            cmpm1 = p_hwk.tile([128, C], F32, tag="cmpm1")
                            nc.vector.tensor_scalar(
                                out=cmpm1[:], in0=lhs[:], scalar1=E2[:, s : s + 1],
                                op0=OP.is_lt, scalar2=-1.0, op1=OP.add)
                            ml2 = p_hwk.tile([128, C], F32, tag="ml2")
                            nc.vector.scalar_tensor_tensor(
                                out=ml2[:], in0=cmpm1[:], scalar=1e9,
                                op0=OP.mult, op1=OP.add, in1=ml[:])
                            mlx = ml2
                        else:
                            mlx = ml

                        nc.scalar.activation(
                            outsb[:, s, 1 : C + 1], mlx[:], AF.Exp,
                            accum_out=rows4[:, s : s + 1])

                    # batched epilogue
                    z0sb = p_sml.tile([128, NSUB, 2], F32, tag="z0sb")
                    nc.scalar.activation(
                        z0sb[:].rearrange("p a b -> p (a b)"), pl2x[:], AF.Exp)
                    col0m = p_sml.tile([128, NSUB], F32, tag="col0m")
                    nc.vector.tensor_scalar(
                        out=col0m[:], in0=mscsb[:, :, 2], scalar1=0.0,
                        op0=OP.not_equal, scalar2=float(eb0), op1=OP.mult)
                    zm0s = p_sml.tile([128, NSUB], F32, tag="zm0s")
                    nc.vector.tensor_tensor(
                        out=zm0s[:], in0=z0sb[:, :, 0], in1=col0m[:], op=OP.mult)
                    tots = p_sml.tile([128, NSUB], F32, tag="tots")
                    nc.vector.tensor_tensor(
                        out=tots[:], in0=rows4[:], in1=zm0s[:], op=OP.add)
                    recs = p_sml.tile([128, NSUB], F32, tag="recs")
                    nc.vector.reciprocal(recs[:], tots[:])
                    nc.vector.tensor_tensor(
                        out=outsb[:, :, 0], in0=zm0s[:], in1=recs[:], op=OP.mult)
                    dens = p_sml.tile([128, NSUB], F32, tag="dens")
                    nc.vector.tensor_scalar(
                        out=dens[:], in0=z0sb[:, :, 1], scalar1=float(embm),
                        op0=OP.mult, scalar2=1.0, op1=OP.add)
                    maintsb = p_sml.tile([128, NSUB], F32, tag="maint")
                    nc.vector.reciprocal(maintsb[:], dens[:])
                    for s in range(NSUB):
                        if s % 2 == 0:
                            nc.scalar.activation(
                                outsb[:, s, 1 : C + 1], outsb[:, s, 1 : C + 1],
                                AF.Copy, scale=recs[:, s : s + 1])
                        else:
                            nc.vector.tensor_scalar(
                                out=outsb[:, s, 1 : C + 1],
                                in0=outsb[:, s, 1 : C + 1],
                                scalar1=recs[:, s : s + 1], scalar2=None,
                                op0=OP.mult)

                    # stores on SWDGE (keep the Sync FIFO free for loads)
                    nc.gpsimd.dma_start(probs_d.ap()[:, t, :, :], outsb[:])
                    nc.gpsimd.dma_start(maint_d.ap()[:, t, :], maintsb[:])

                # prologue: prefetch 2 tiles, trunk of tile 0
                emit_loads(0)
                emit_loads(1)
                emit_h1(0)
                emit_h2(0)
                for t in range(1, NT):
                    emit_loads(t + 1) if t + 1 < NT else None
                    emit_h1(t)
                    emit_heads(t - 1)        # overlaps with tile t's trunk
                    emit_h2(t)
                emit_heads(NT - 1)

    nc.compile()
    return nc


def kernel(x, y, d, uav_obs, W1, b1, W2, b2, Wm, bm, Wr, br,
           demands, dist, max_load, fix_v, energy_param, max_energy):
    x = np.asarray(x, dtype=np.float32)
    y = np.asarray(y, dtype=np.float32)
    d = np.asarray(d, dtype=np.float32)
    uav_obs = np.asarray(uav_obs, dtype=np.float32)
    W1 = np.asarray(W1, dtype=np.float32)
    b1 = np.asarray(b1, dtype=np.float32)
    W2 = np.asarray(W2, dtype=np.float32)
    b2 = np.asarray(b2, dtype=np.float32)
    Wm = np.asarray(Wm, dtype=np.float32)
    bm = np.asarray(bm, dtype=np.float32)
    Wr = np.asarray(Wr, dtype=np.float32)
    br = np.asarray(br, dtype=np.float32)
    demands = np.asarray(demands, dtype=np.float32)
    dist = np.asarray(dist, dtype=np.float32)
    max_load = np.float32(max_load)
    fix_v = np.float32(fix_v)
    energy_param = np.float32(energy_param)
    max_energy = np.float32(max_energy)
    bf16 = ml_dtypes.bfloat16

    load_full = uav_obs[:, 3]
    energy_full = uav_obs[:, 4]
    dest_full = uav_obs[:, 1]

    # s-table (same fp32 rounding as the reference) and threshold constants
    arr_wh = dist[1:, 0] / fix_v
    tab = dist[:, 1:] / fix_v + arr_wh[None, :]            # [513, 512]
    c1 = float(energy_param) * 0.1 / float(max_energy)
    if not (c1 > 0.0 and np.isfinite(c1)):
        raise NotImplementedError("nonpositive energy cost coefficient")
    inv_c1sq = np.float32(1.0 / (c1 * c1))
    assert (energy_full >= 0).all() and (tab >= 0).all()
    assert load_full.max() + demands.max() <= float(max_load), \
        "load clause not always true; unsupported fast path"

    # rows that can possibly be energy-infeasible somewhere
    sup_m = (c1 * (100.0 + float(load_full.max()) + float(demands.max())) ** 1.5
             * float(tab.max()) * (1.0 + 1e-5) + 1e-30)
    is_hard = energy_full <= sup_m

    # per-core permutation: hard rows first, padded to whole 512-row tiles
    perms = []
    n_hard_tiles = 0
    for c in range(N_CORES):
        rs = slice(c * BSH, (c + 1) * BSH)
        flag = is_hard[rs]
        perm = np.argsort(~flag, kind="stable")            # hard first
        perms.append(perm)
        n_hard_tiles = max(n_hard_tiles, int(-(-int(flag.sum()) // 512)))

    # stay/maint biases folded into the epilogue constants
    eb0 = float(np.exp(np.float64(br[0])))
    embm = float(np.exp(-np.float64(bm[0])))
    key = (float(inv_c1sq), n_hard_tiles, eb0, embm)
    if key not in _PROG_CACHE:
        _PROG_CACHE[key] = _build_program(inv_c1sq, n_hard_tiles, eb0, embm)
    nc = _PROG_CACHE[key]

    # head weights: [customer logits 1..512 | stay logit | -maint logit]
    wrm = np.concatenate([Wr[:, 1:], Wr[:, 0:1], -Wm], axis=1).astype(bf16)

    common = {
        "tab": tab,
        "w1": W1.astype(bf16),
        "w2": W2.astype(bf16),
        "wrm": wrm,
        "dem": demands.reshape(1, C),
        "b1v": b1.reshape(128, 1),
        "b2v": b2.reshape(128, 1),
    }

    ht_full = np.concatenate([x, y, d, uav_obs], axis=1)   # [B, DIN] f32
    seq_full = uav_obs[:, OBS - C :]
    dest_idx16 = dest_full.astype(np.int16)

    in_maps = []
    for c in range(N_CORES):
        rs = slice(c * BSH, (c + 1) * BSH)
        perm = perms[c]
        htp = ht_full[rs][perm]                            # [BSH, DIN] f32
        seqp = seq_full[rs][perm]
        # big: [128, NT, 20, 512] bf16 -- ht chunks 0..15 + seqpen planes
        big = np.empty((128, NT, NBIG, 512), dtype=bf16)
        htT = np.ascontiguousarray(htp.T).astype(bf16)     # [DIN, BSH]
        big[:, :, 0:KFULL, :] = (
            htT[0 : KFULL * 128]
            .reshape(KFULL, 128, NT, 512)
            .transpose(1, 2, 0, 3)
        )
        seqpen = ((seqp.astype(np.float32) - 1.0) * np.float32(1e9)
                  + br[None, 1:]).astype(bf16)             # br_j or ~-1e9
        big[:, :, KFULL:NBIG, :] = (
            seqpen.reshape(NT, NSUB, 128, C).transpose(2, 0, 1, 3)
        )
        ht7 = np.ascontiguousarray(
            htT[KFULL * 128 :].reshape(KREM, NT, 512))
        msc = np.zeros((128, NT, NSUB, 4), np.float32)
        uavp = uav_obs[rs][perm]
        lep = np.stack([uavp[:, 3], uavp[:, 4], uavp[:, 1]], axis=-1)
        msc[:, :, :, 0:3] = lep.reshape(NT, NSUB, 128, 3).transpose(2, 0, 1, 3)
        nh = max(n_hard_tiles, 1)
        idxp = dest_idx16[rs][perm][: nh * 512]
        didx = np.ascontiguousarray(
            np.tile(idxp.reshape(nh * 32, 16).T, (8, 1)))  # [128, nh*32]
        in_maps.append(dict(common, big=big, ht7=ht7, msc=msc, didx=didx))

    trace = os.environ.get("BASS_KERNEL_TRACE", "0") == "1"
    res = run_bass_kernel_spmd(nc, in_maps, list(range(N_CORES)), trace=trace)
    global LAST_RESULT
    LAST_RESULT = res

    maintenance = np.empty((B, 1), np.float32)
    probs = np.empty((B, C + 1), np.float32)
    for c in range(N_CORES):
        rs = slice(c * BSH, (c + 1) * BSH)
        pm = res.results[c]["probs"]                       # [128, NT, 4, 513]
        mm = res.results[c]["maint"]                       # [128, NT, 4]
        pr = pm.transpose(1, 2, 0, 3).reshape(BSH, C + 1)
        mt = mm.transpose(1, 2, 0).reshape(BSH)
        probs[rs.start + perms[c]] = pr
        maintenance[rs.start + perms[c], 0] = mt
    return maintenance, probs


# revision 22
# speedup vs baseline: 1.2006x; 1.2006x over previous
"""Trainium2 Bass kernel for nn_ActorNetwork (topk_masking).

Pure data-parallel over 8 NeuronCores: each core processes 8192 rows of the
65536-row batch (512-row tiles, 4x128-row subtiles).

Math per row b, customer j (C=512):
  h1 = relu(h @ W1 + b1), h2 = relu(h1 @ W2 + b2)      h = [x|y|d|uav]
  logits = h2 @ Wr + br; maintenance = sigmoid(h2 @ Wm + bm)
  mask: col0 = dest != 0;  col 1+j = seq_j & (load+dem_j <= ML) & (e_wh > 0)
    e_wh = energy - cost*s,  cost = ((ep*x^1.5)*0.1)/ME,
    x = (100+load)+dem_j,  s = dist[dest,1+j]/fv + dist[1+j,0]/fv
  probs = softmax(where(mask, logits, -1e9))

Device strategy:
  * MLP in transposed layout (bf16 matmuls, fp32 psum), heads in natural
    layout; biases via per-partition ACT bias (b1,b2) and K=1 matmuls (br).
  * masking via additive -1e9 penalties folded into logits before exp.
  * feasibility: rows with energy > sup(cost*s) (fp64 bound) are feasible at
    every j -- the host groups the few "hard" rows into leading tiles; only
    those run the exact fp32 squared comparison
        s^2 * x^3 < energy^2 / (ep*0.1/ME)^2
    (multiplies only -- bit-faithful to ~3ulp of the reference boundary)
    with s gathered from a host-precomputed table via dma_gather.
  * softmax without max-subtraction (logits are O(0.1) by construction);
    exp and row-sum fused in one ACT op (accum_out); 1/sum scale applied
    by ACT Copy with per-partition scale. Only the exp_and_others ACT
    table set is used (sigmoid computed as 1/(1+exp(-v))).

Host does only relabeling (transpose/cast/permute/concat) plus O(C^2)
constant tables and O(B) row classification.
"""

import os
import sys

for _p in ("/opt/trn_rl_repo", "/root/.axon_site/_ro/trn_rl_repo", "/root/.axon_site"):
    if os.path.isdir(_p) and _p not in sys.path:
        sys.path.append(_p)

os.environ.setdefault("JAX_PLATFORMS", "axon")

import numpy as np
import ml_dtypes

import concourse.bass as bass
import concourse.mybir as mybir
import concourse.tile as tile
from concourse import bacc
from concourse.bass import ts
from concourse.bass_utils import run_bass_kernel_spmd

F32 = mybir.dt.float32
BF16 = mybir.dt.bfloat16
I16 = mybir.dt.int16
AF = mybir.ActivationFunctionType
OP = mybir.AluOpType

N_CORES = 8
B = 65536
C = 512
OBS = 519
DIN = OBS + 3 * C            # 2055
BSH = B // N_CORES           # 8192
NT = BSH // 512              # 16 tiles of 512 rows
KFULL = DIN // 128           # 16 full K-chunks
KREM = DIN - KFULL * 128     # 7
NSUB = 4                     # 4 subtiles of 128 rows per tile
NBIG = KFULL + NSUB          # 20: ht chunks + seqpen planes in one buffer

_PROG_CACHE = {}
LAST_RESULT = None


def _build_program(inv_c1sq, n_hard, eb0, embm):
    nc = bacc.Bacc("TRN2", target_bir_lowering=False, debug=False)

    big_d = nc.dram_tensor("big", [128, NT, NBIG, 512], BF16, kind="ExternalInput")
    ht7_d = nc.dram_tensor("ht7", [KREM, NT, 512], BF16, kind="ExternalInput")
    msc_d = nc.dram_tensor("msc", [128, NT, NSUB, 4], F32, kind="ExternalInput")
    didx_d = nc.dram_tensor("didx", [128, max(n_hard, 1) * 32], I16,
                            kind="ExternalInput")
    tab_d = nc.dram_tensor("tab", [C + 1, C], F32, kind="ExternalInput")
    w1_d = nc.dram_tensor("w1", [DIN, 128], BF16, kind="ExternalInput")
    w2_d = nc.dram_tensor("w2", [128, 128], BF16, kind="ExternalInput")
    wrm_d = nc.dram_tensor("wrm", [128, C + 2], BF16, kind="ExternalInput")
    dem_d = nc.dram_tensor("dem", [1, C], F32, kind="ExternalInput")
    b1_d = nc.dram_tensor("b1v", [128, 1], F32, kind="ExternalInput")
    b2_d = nc.dram_tensor("b2v", [128, 1], F32, kind="ExternalInput")

    probs_d = nc.dram_tensor("probs", [128, NT, NSUB, C + 1], F32,
                             kind="ExternalOutput")
    maint_d = nc.dram_tensor("maint", [128, NT, NSUB], F32, kind="ExternalOutput")

    with tile.TileContext(nc) as tc:
        with tc.tile_pool(name="const", bufs=1) as cpool:
            w1sb = cpool.tile([128, KFULL + 1, 128], BF16)
            nc.sync.dma_start(
                w1sb[:, 0:KFULL, :],
                w1_d.ap()[0 : KFULL * 128, :].rearrange("(k p) m -> p k m", p=128),
            )
            nc.sync.dma_start(w1sb[0:KREM, KFULL, :], w1_d.ap()[KFULL * 128 :, :])
            w2sb = cpool.tile([128, 128], BF16)
            nc.sync.dma_start(w2sb[:], w2_d.ap())
            wrmsb = cpool.tile([128, C + 2], BF16)
            nc.sync.dma_start(wrmsb[:], wrm_d.ap())
            b1sb = cpool.tile([128, 1], F32)
            nc.sync.dma_start(b1sb[:], b1_d.ap())
            b2sb = cpool.tile([128, 1], F32)
            nc.sync.dma_start(b2sb[:], b2_d.ap())
            dem1 = cpool.tile([1, C], F32)
            nc.sync.dma_start(dem1[:], dem_d.ap())
            demb = cpool.tile([128, C], F32)
            nc.gpsimd.partition_broadcast(demb[:], dem1[0:1, :])
            didxsb = cpool.tile([128, max(n_hard, 1) * 32], I16)
            nc.sync.dma_start(didxsb[:], didx_d.ap())

            with (
                tc.tile_pool(name="big", bufs=4) as p_big,
                tc.tile_pool(name="ht7", bufs=4) as p_ht7,
                tc.tile_pool(name="msc", bufs=4) as p_msc,
                tc.tile_pool(name="gat", bufs=1) as p_gat,
                tc.tile_pool(name="h12", bufs=3) as p_h12,
                tc.tile_pool(name="wrk", bufs=2) as p_wrk,
                tc.tile_pool(name="hwk", bufs=1) as p_hwk,
                tc.tile_pool(name="sml", bufs=3) as p_sml,
                tc.tile_pool(name="out", bufs=3) as p_out,
                tc.tile_pool(name="psA", bufs=3, space="PSUM") as psA,
                tc.tile_pool(name="psL", bufs=3, space="PSUM") as psL,
                tc.tile_pool(name="psS", bufs=2, space="PSUM") as psS,
            ):
                for t in range(NT):
                    hard = t < n_hard
                    # ------- loads (hoisted for the scheduler) -------
                    with tc.high_priority(offset=200):
                        bigsb = p_big.tile([128, NBIG, 512], BF16, tag="big")
                        nc.sync.dma_start(bigsb[:], big_d.ap()[:, t, :, :])
                        ht7sb = p_ht7.tile([128, 512], BF16, tag="ht7")
                        nc.sync.dma_start(ht7sb[0:KREM, :], ht7_d.ap()[:, t, :])
                        mscsb = p_msc.tile([128, NSUB, 4], F32, tag="msc")
                        nc.sync.dma_start(mscsb[:], msc_d.ap()[:, t, :, :])
                        if hard:
                            ssb = p_gat.tile([128, NSUB, C], F32, tag="gat")
                            nc.gpsimd.dma_gather(
                                out_ap=ssb[:], in_ap=tab_d.ap(),
                                idxs_ap=didxsb[:, ts(t, 32)],
                                num_idxs=512, num_idxs_reg=512, elem_size=C,
                            )

                    # ------- MLP trunk (transposed layout) -------
                    ph1 = psA.tile([128, 512], F32, tag="psA")
                    for k in range(KFULL):
                        nc.tensor.matmul(ph1[:], w1sb[:, k, :], bigsb[:, k, :],
                                         start=(k == 0), stop=False)
                    nc.tensor.matmul(ph1[:], w1sb[0:KREM, KFULL, :],
                                     ht7sb[0:KREM, :], start=False, stop=True)
                    h1sb = p_h12.tile([128, 512], BF16, tag="h12")
                    with tc.high_priority(offset=60):
                        nc.scalar.activation(h1sb[:], ph1[:], AF.Relu,
                                             bias=b1sb[:], scale=1.0)
                    ph2 = psA.tile([128, 512], F32, tag="psA")
                    nc.tensor.matmul(ph2[:], w2sb[:], h1sb[:], start=True, stop=True)
                    h2sb = p_h12.tile([128, 512], BF16, tag="h12")
                    with tc.high_priority(offset=60):
                        nc.scalar.activation(h2sb[:], ph2[:], AF.Relu,
                                             bias=b2sb[:], scale=1.0)

                    outsb = p_out.tile([128, NSUB, C + 1], F32, tag="out")
                    rows4 = p_sml.tile([128, NSUB], F32, tag="rows4")
                    pl2x = psS.tile([128, 2 * NSUB], F32, tag="psS")

                    if hard:
                        l100 = p_sml.tile([128, NSUB], F32, tag="l100")
                        nc.vector.tensor_scalar(
                            out=l100[:], in0=mscsb[:, :, 0], scalar1=100.0,
                            scalar2=None, op0=OP.add)
                        e2t = p_sml.tile([128, NSUB], F32, tag="e2t")
                        nc.vector.tensor_tensor(
                            out=e2t[:], in0=mscsb[:, :, 1], in1=mscsb[:, :, 1],
                            op=OP.mult)
                        E2 = p_sml.tile([128, NSUB], F32, tag="E2")
                        nc.vector.tensor_scalar(
                            out=E2[:], in0=e2t[:], scalar1=float(inv_c1sq),
                            scalar2=None, op0=OP.mult)

                    for s in range(NSUB):
                        # ------- heads (br carried by the penalty plane) -------
                        pl = psL.tile([128, 512], F32, tag="psL")
                        nc.tensor.matmul(pl[:], h2sb[:, ts(s, 128)],
                                         wrmsb[:, 0:512], start=True, stop=True)
                        nc.tensor.matmul(pl2x[:, 2 * s : 2 * s + 2],
                                         h2sb[:, ts(s, 128)],
                                         wrmsb[:, 512:514], start=True, stop=True)

                        # logits + seq penalty (-1e9 where seq==0, + br)
                        ml = p_wrk.tile([128, C], F32, tag="ml")
                        nc.vector.tensor_tensor(
                            out=ml[:], in0=pl[:], in1=bigsb[:, KFULL + s, :],
                            op=OP.add)

                        if hard:
                            xb = p_hwk.tile([128, C], F32, tag="xb")
                            nc.vector.tensor_scalar(
                                out=xb[:], in0=demb[:], scalar1=l100[:, s : s + 1],
                                scalar2=None, op0=OP.add)
                            sx = p_hwk.tile([128, C], F32, tag="sx")
                            nc.vector.tensor_tensor(
                                out=sx[:], in0=ssb[:, s, :], in1=xb[:], op=OP.mult)
                            sx2 = p_hwk.tile([128, C], F32, tag="sx2")
                            nc.scalar.activation(sx2[:], sx[:], AF.Square)
                            lhs = p_hwk.tile([128, C], F32, tag="lhs")
                            nc.gpsimd.tensor_tensor(
                                out=lhs[:], in0=sx2[:], in1=xb[:], op=OP.mult)
                            cmpm1 = p_hwk.tile([128, C], F32, tag="cmpm1")
                            nc.vector.tensor_scalar(
                                out=cmpm1[:], in0=lhs[:], scalar1=E2[:, s : s + 1],
                                op0=OP.is_lt, scalar2=-1.0, op1=OP.add)
                            ml2 = p_hwk.tile([128, C], F32, tag="ml2")
                            nc.vector.scalar_tensor_tensor(
                                out=ml2[:], in0=cmpm1[:], scalar=1e9,
                                op0=OP.mult, op1=OP.add, in1=ml[:])
                            mlx = ml2
                        else:
                            mlx = ml

                        # exp + row-sum fused
                        nc.scalar.activation(
                            outsb[:, s, 1 : C + 1], mlx[:], AF.Exp,
                            accum_out=rows4[:, s : s + 1])

                    # ------- batched epilogue -------
                    z0sb = p_sml.tile([128, NSUB, 2], F32, tag="z0sb")
                    nc.scalar.activation(
                        z0sb[:].rearrange("p a b -> p (a b)"), pl2x[:], AF.Exp)
                    col0m = p_sml.tile([128, NSUB], F32, tag="col0m")
                    nc.vector.tensor_scalar(
                        out=col0m[:], in0=mscsb[:, :, 2], scalar1=0.0,
                        op0=OP.not_equal, scalar2=float(eb0), op1=OP.mult)
                    zm0s = p_sml.tile([128, NSUB], F32, tag="zm0s")
                    nc.vector.tensor_tensor(
                        out=zm0s[:], in0=z0sb[:, :, 0], in1=col0m[:], op=OP.mult)
                    tots = p_sml.tile([128, NSUB], F32, tag="tots")
                    nc.vector.tensor_tensor(
                        out=tots[:], in0=rows4[:], in1=zm0s[:], op=OP.add)
                    recs = p_sml.tile([128, NSUB], F32, tag="recs")
                    nc.vector.reciprocal(recs[:], tots[:])
                    nc.vector.tensor_tensor(
                        out=outsb[:, :, 0], in0=zm0s[:], in1=recs[:], op=OP.mult)
                    dens = p_sml.tile([128, NSUB], F32, tag="dens")
                    nc.vector.tensor_scalar(
                        out=dens[:], in0=z0sb[:, :, 1], scalar1=float(embm),
                        op0=OP.mult, scalar2=1.0, op1=OP.add)
                    maintsb = p_sml.tile([128, NSUB], F32, tag="maint")
                    nc.vector.reciprocal(maintsb[:], dens[:])
                    for s in range(NSUB):
                        if s % 2 == 0:
                            nc.scalar.activation(
                                outsb[:, s, 1 : C + 1], outsb[:, s, 1 : C + 1],
                                AF.Copy, scale=recs[:, s : s + 1])
                        else:
                            nc.vector.tensor_scalar(
                                out=outsb[:, s, 1 : C + 1],
                                in0=outsb[:, s, 1 : C + 1],
                                scalar1=recs[:, s : s + 1], scalar2=None,
                                op0=OP.mult)

                    # ------- stores (SWDGE: keep the Sync FIFO free for loads) ---
                    nc.gpsimd.dma_start(probs_d.ap()[:, t, :, :], outsb[:])
                    nc.gpsimd.dma_start(maint_d.ap()[:, t, :], maintsb[:])

    nc.compile()
    return nc


def kernel(x, y, d, uav_obs, W1, b1, W2, b2, Wm, bm, Wr, br,
           demands, dist, max_load, fix_v, energy_param, max_energy):
    x = np.asarray(x, dtype=np.float32)
    y = np.asarray(y, dtype=np.float32)
    d = np.asarray(d, dtype=np.float32)
    uav_obs = np.asarray(uav_obs, dtype=np.float32)
    W1 = np.asarray(W1, dtype=np.float32)
    b1 = np.asarray(b1, dtype=np.float32)
    W2 = np.asarray(W2, dtype=np.float32)
    b2 = np.asarray(b2, dtype=np.float32)
    Wm = np.asarray(Wm, dtype=np.float32)
    bm = np.asarray(bm, dtype=np.float32)
    Wr = np.asarray(Wr, dtype=np.float32)
    br = np.asarray(br, dtype=np.float32)
    demands = np.asarray(demands, dtype=np.float32)
    dist = np.asarray(dist, dtype=np.float32)
    max_load = np.float32(max_load)
    fix_v = np.float32(fix_v)
    energy_param = np.float32(energy_param)
    max_energy = np.float32(max_energy)
    bf16 = ml_dtypes.bfloat16

    load_full = uav_obs[:, 3]
    energy_full = uav_obs[:, 4]
    dest_full = uav_obs[:, 1]

    # s-table (same fp32 rounding as the reference) and threshold constants
    arr_wh = dist[1:, 0] / fix_v
    tab = dist[:, 1:] / fix_v + arr_wh[None, :]            # [513, 512]
    c1 = float(energy_param) * 0.1 / float(max_energy)
    if not (c1 > 0.0 and np.isfinite(c1)):
        raise NotImplementedError("nonpositive energy cost coefficient")
    inv_c1sq = np.float32(1.0 / (c1 * c1))
    assert (energy_full >= 0).all() and (tab >= 0).all()
    assert load_full.max() + demands.max() <= float(max_load), \
        "load clause not always true; unsupported fast path"

    # rows that can possibly be energy-infeasible somewhere
    sup_m = (c1 * (100.0 + float(load_full.max()) + float(demands.max())) ** 1.5
             * float(tab.max()) * (1.0 + 1e-5) + 1e-30)
    is_hard = energy_full <= sup_m

    # per-core permutation: hard rows first, padded to whole 512-row tiles
    perms = []
    n_hard_tiles = 0
    for c in range(N_CORES):
        rs = slice(c * BSH, (c + 1) * BSH)
        flag = is_hard[rs]
        perm = np.argsort(~flag, kind="stable")            # hard first
        perms.append(perm)
        n_hard_tiles = max(n_hard_tiles, int(-(-int(flag.sum()) // 512)))

    # stay/maint biases folded into the epilogue constants
    eb0 = float(np.exp(np.float64(br[0])))
    embm = float(np.exp(-np.float64(bm[0])))
    key = (float(inv_c1sq), n_hard_tiles, eb0, embm)
    if key not in _PROG_CACHE:
        _PROG_CACHE[key] = _build_program(inv_c1sq, n_hard_tiles, eb0, embm)
    nc = _PROG_CACHE[key]

    # head weights: [customer logits 1..512 | stay logit | -maint logit]
    wrm = np.concatenate([Wr[:, 1:], Wr[:, 0:1], -Wm], axis=1).astype(bf16)

    common = {
        "tab": tab,
        "w1": W1.astype(bf16),
        "w2": W2.astype(bf16),
        "wrm": wrm,
        "dem": demands.reshape(1, C),
        "b1v": b1.reshape(128, 1),
        "b2v": b2.reshape(128, 1),
    }

    ht_full = np.concatenate([x, y, d, uav_obs], axis=1)   # [B, DIN] f32
    seq_full = uav_obs[:, OBS - C :]
    dest_idx16 = dest_full.astype(np.int16)

    in_maps = []
    for c in range(N_CORES):
        rs = slice(c * BSH, (c + 1) * BSH)
        perm = perms[c]
        htp = ht_full[rs][perm]                            # [BSH, DIN] f32
        seqp = seq_full[rs][perm]
        # big: [128, NT, 20, 512] bf16 -- ht chunks 0..15 + seqpen planes
        big = np.empty((128, NT, NBIG, 512), dtype=bf16)
        htT = np.ascontiguousarray(htp.T).astype(bf16)     # [DIN, BSH]
        big[:, :, 0:KFULL, :] = (
            htT[0 : KFULL * 128]
            .reshape(KFULL, 128, NT, 512)
            .transpose(1, 2, 0, 3)
        )
        seqpen = ((seqp.astype(np.float32) - 1.0) * np.float32(1e9)
                  + br[None, 1:]).astype(bf16)             # br_j or ~-1e9
        big[:, :, KFULL:NBIG, :] = (
            seqpen.reshape(NT, NSUB, 128, C).transpose(2, 0, 1, 3)
        )
        ht7 = np.ascontiguousarray(
            htT[KFULL * 128 :].reshape(KREM, NT, 512))
        msc = np.zeros((128, NT, NSUB, 4), np.float32)
        lep = np.stack([uav_obs[rs][perm][:, 3],
                        uav_obs[rs][perm][:, 4],
                        uav_obs[rs][perm][:, 1]], axis=-1)  # load, energy, dest
        msc[:, :, :, 0:3] = lep.reshape(NT, NSUB, 128, 3).transpose(2, 0, 1, 3)
        nh = max(n_hard_tiles, 1)
        idxp = dest_idx16[rs][perm][: nh * 512]
        didx = np.ascontiguousarray(
            np.tile(idxp.reshape(nh * 32, 16).T, (8, 1)))  # [128, nh*32]
        in_maps.append(dict(common, big=big, ht7=ht7, msc=msc, didx=didx))

    trace = os.environ.get("BASS_KERNEL_TRACE", "0") == "1"
    res = run_bass_kernel_spmd(nc, in_maps, list(range(N_CORES)), trace=trace)
    global LAST_RESULT
    LAST_RESULT = res

    maintenance = np.empty((B, 1), np.float32)
    probs = np.empty((B, C + 1), np.float32)
    for c in range(N_CORES):
        rs = slice(c * BSH, (c + 1) * BSH)
        pm = res.results[c]["probs"]                       # [128, NT, 4, 513]
        mm = res.results[c]["maint"]                       # [128, NT, 4]
        pr = pm.transpose(1, 2, 0, 3).reshape(BSH, C + 1)
        mt = mm.transpose(1, 2, 0).reshape(BSH)
        inv = perms[c]
        probs[rs.start + inv] = pr
        maintenance[rs.start + inv, 0] = mt
    return maintenance, probs
